# revision 7
# baseline (speedup 1.0000x reference)
"""BLT model TRN2 kernel — nn_BLTModel_13872744366807.

Strategy:
- Vocab collapse: the byte-axis path (embedding -> CA query -> CA output ->
  logits) depends only on byte VALUE (V=256) and batch, so the [B,4096,*]
  byte axis collapses to a [B,256,*] table; patch mean-pooling becomes a
  host-computed histogram matrix times emb; final output is a host gather.
- Device: 4-layer global transformer on [512, 1024] patch tokens,
  Megatron TP-8 (heads/hidden sharded), feature-major activations,
  fp32r matmuls, LayerNorm commuted through weight matmuls (affines
  host-folded, colsum fixups — exact), fp16 AllReduce payloads (8x) plus
  one f32 AllReduce for the collapsed CA/head partials.
"""
import numpy as np
import concourse.bacc as bacc
import concourse.bass as bass
import concourse.mybir as mybir
from concourse import tile
from concourse.bass_utils import run_bass_kernel_spmd
from concourse.bass_interp import get_hw_module

F32 = mybir.dt.float32
F32R = mybir.dt.float32r
FP16 = mybir.dt.float16
AF = mybir.ActivationFunctionType
ALU = mybir.AluOpType

L, B, S, P, H, V, NC = 4, 2, 4096, 256, 1024, 256, 8
T = B * P          # 512 tokens through the global transformer
EPS = 1e-6
RG8 = [list(range(NC))]

_CACHE = {}


# --------------------------------------------------------------------------
# device program
# --------------------------------------------------------------------------
def _trace():
    nc = bacc.Bacc("TRN2", target_bir_lowering=False, debug=False,
                   num_devices=NC)
    d = {}

    def inp(name, shape, dt=F32R):
        d[name] = nc.dram_tensor(name, shape, dt, kind="ExternalInput").ap()

    inp("wqkv", [L, 128, 3072])
    inp("wsq", [L, 128, 3], F32)
    inp("ngq", [L, 128, 3], F32)
    inp("wo", [L, 128, 1024])
    inp("bo8", [L, 128, 8], F32)
    inp("w1", [L, 128, 4096])
    inp("ws1", [L, 128, 4], F32)
    inp("ng1", [L, 128, 4], F32)
    inp("w2", [L, 128, 4096])
    inp("b28", [L, 128, 8], F32)
    inp("wq", [128, 1024]); inp("wk", [128, 1024]); inp("wv", [128, 1024])
    inp("bq", [128, 1], F32); inp("bk", [128, 1], F32); inp("bv", [128, 1], F32)
    inp("cawoT", [128, 1024])
    inp("headw", [128, 2048])
    inp("headb", [128, 2], F32)
    inp("embT", [128, 2048])
    inp("embS", [128, 2048])
    inp("cnt", [128, 1024])
    inp("masks", [128, 512])
    inp("ones", [128, 128])
    inp("ident", [128, 128])
    inp("fng", [128, 8], F32); inp("fnb", [128, 8], F32)
    inp("cag", [128, 8], F32); inp("cab", [128, 8], F32)
    out_d = nc.dram_tensor("ltab", [128, 1024], F32, kind="ExternalOutput").ap()

    with tile.TileContext(nc) as tc:
        with (
            tc.tile_pool(name="const", bufs=1) as cp,
            tc.tile_pool(name="sb", bufs=1) as sbp,
            tc.tile_pool(name="wts", bufs=1) as wp,
            tc.tile_pool(name="tmp", bufs=2) as tp,
            tc.tile_pool(name="tps", bufs=1) as tps,
            tc.tile_pool(name="pp", bufs=3, space="PSUM") as pp,
            tc.tile_pool(name="pa", bufs=2, space="PSUM") as pa,
            tc.tile_pool(name="pst", bufs=2, space="PSUM") as pst,
            tc.tile_pool(name="dram", bufs=1, space="DRAM") as dp,
        ):
            # ---------------- constants ----------------
            def cload(name, shape, dt=F32R):
                t_ = cp.tile(shape, dt, tag=name)
                nc.sync.dma_start(t_[:], d[name][:])
                return t_

            ones_t = cload("ones", [128, 128])
            ident_t = cload("ident", [128, 128])
            masks_t = cload("masks", [128, 512])
            fng_t = cload("fng", [128, 8], F32); fnb_t = cload("fnb", [128, 8], F32)
            cag_t = cload("cag", [128, 8], F32); cab_t = cload("cab", [128, 8], F32)
            headb_t = cload("headb", [128, 2], F32)
            bq_t = cload("bq", [128, 1], F32); bk_t = cload("bk", [128, 1], F32)
            bv_t = cload("bv", [128, 1], F32)
            embS_t = wp.tile([128, 2, 1024], F32R, tag="w2")
            nc.sync.dma_start(embS_t[:], d["embS"][:].rearrange(
                "p (vc x) -> p vc x", vc=2))
            cnt_t = wp.tile([128, 2, 512], F32R, tag="w1")
            nc.sync.dma_start(cnt_t[:], d["cnt"][:].rearrange(
                "p (vc x) -> p vc x", vc=2))

            # ---------------- persistent activations ----------------
            h_t = sbp.tile([128, 8, 512], F32R, tag="h")
            sq_t = sbp.tile([128, 8, 512], F32R, tag="sq")
            qkv_t = sbp.tile([128, 3, 512], F32R, tag="qkv")
            qkvh2_t = sbp.tile([64, 3, 512], F32R, tag="qkvh2")
            A_t = sbp.tile([128, 512], F32R, tag="A")
            gu_t = sbp.tile([128, 4, 512], F32R, tag="gu")
            aro_t = sbp.tile([128, 8, 512], FP16, tag="aro")
            ari_t = sbp.tile([128, 8, 512], FP16, tag="ari")

            # ---------------- helpers ----------------
            def stats(src, n8, width):
                """src: [128, n8, width] f32r. Returns (rsig_b, musig_b)
                [128, width] f32r sbuf tiles (broadcast along partitions)."""
                for ti in range(n8):
                    nc.scalar.activation(sq_t[:, ti, :width], src[:, ti, :width],
                                         AF.Square)
                ps_sum = pst.tile([1, 512], F32, tag="stat")
                ps_sq = pst.tile([1, 512], F32, tag="stat")
                for ti in range(n8):
                    nc.tensor.matmul(ps_sum[:, :width], ones_t[:, 0:1],
                                     src[:, ti, :width],
                                     start=(ti == 0), stop=(ti == n8 - 1))
                for ti in range(n8):
                    nc.tensor.matmul(ps_sq[:, :width], ones_t[:, 0:1],
                                     sq_t[:, ti, :width],
                                     start=(ti == 0), stop=(ti == n8 - 1))
                inv = 1.0 / (n8 * 128)
                mu = tps.tile([1, 512], F32R, tag="mu")
                nc.vector.tensor_scalar_mul(mu[:, :width], ps_sum[:, :width], inv)
                ex2 = tps.tile([1, 512], F32R, tag="ex2")
                nc.vector.tensor_scalar(out=ex2[:, :width], in0=ps_sq[:, :width],
                                        scalar1=inv, scalar2=EPS,
                                        op0=ALU.mult, op1=ALU.add)
                mus = tps.tile([1, 512], F32R, tag="mus")
                nc.scalar.activation(mus[:, :width], mu[:, :width], AF.Square)
                vare = tps.tile([1, 512], F32R, tag="var")
                nc.vector.tensor_tensor(out=vare[:, :width], in0=ex2[:, :width],
                                        in1=mus[:, :width], op=ALU.subtract)
                vrec = tps.tile([1, 512], F32R, tag="vrec")
                with nc.allow_low_precision(reason="f32r is full-width here"):
                    nc.vector.reciprocal(vrec[:, :width], vare[:, :width])
                rsig = tps.tile([1, 512], F32R, tag="rsig")
                nc.scalar.activation(rsig[:, :width], vrec[:, :width], AF.Sqrt)
                musg = tps.tile([1, 512], F32R, tag="musg")
                nc.vector.tensor_tensor(out=musg[:, :width], in0=mu[:, :width],
                                        in1=rsig[:, :width], op=ALU.mult)
                pb = pp.tile([128, 512], F32, tag="mm")
                nc.tensor.matmul(pb[:, :width], ones_t[0:1, :], rsig[:, :width],
                                 start=True, stop=True)
                rsig_b = tp.tile([128, 512], F32R, tag="rsigb")
                nc.vector.tensor_copy(rsig_b[:, :width], pb[:, :width])
                pb2 = pp.tile([128, 512], F32, tag="mm")
                nc.tensor.matmul(pb2[:, :width], ones_t[0:1, :], musg[:, :width],
                                 start=True, stop=True)
                musig_b = tp.tile([128, 512], F32R, tag="musgb")
                nc.vector.tensor_copy(musig_b[:, :width], pb2[:, :width])
                return rsig_b, musig_b

            def fixup(ps, mcol, rsig_b, musig_b, wsum_t, negb_t, out_ap,
                      gelu=False):
                """out = ps*rsig_b - (musig_b*wsum - (-negb)); optional Gelu."""
                t1 = tp.tile([128, 512], F32R, tag="fx1")
                nc.vector.tensor_tensor(out=t1[:], in0=ps[:], in1=rsig_b[:],
                                        op=ALU.mult)
                m2 = tp.tile([128, 512], F32R, tag="fx2")
                nc.vector.tensor_scalar(out=m2[:], in0=musig_b[:],
                                        scalar1=wsum_t[:, mcol:mcol + 1],
                                        scalar2=negb_t[:, mcol:mcol + 1],
                                        op0=ALU.mult, op1=ALU.add)
                if gelu:
                    t2 = tp.tile([128, 512], F32R, tag="fx3")
                    nc.vector.tensor_tensor(out=t2[:], in0=t1[:], in1=m2[:],
                                            op=ALU.subtract)
                    nc.scalar.activation(out_ap, t2[:], AF.Gelu)
                else:
                    nc.vector.tensor_tensor(out=out_ap, in0=t1[:], in1=m2[:],
                                            op=ALU.subtract)

            def allreduce_fp16(tag):
                bin_ = dp.tile([128, 4096], FP16, tag=f"ci{tag}")
                bout = dp.tile([128, 4096], FP16, addr_space="Shared",
                               tag=f"co{tag}")
                for q in range(4):
                    nc.sync.dma_start(bin_[:, q * 1024:(q + 1) * 1024],
                                      aro_t[:, q * 2:(q + 1) * 2, :])
                nc.gpsimd.collective_compute(
                    "AllReduce", ALU.add, replica_groups=RG8,
                    ins=[bin_[:].opt()], outs=[bout[:].opt()])
                for q in range(4):
                    nc.sync.dma_start(ari_t[:, q * 2:(q + 1) * 2, :],
                                      bout[:, q * 1024:(q + 1) * 1024])

            def resid_add():
                for ti in range(8):
                    nc.vector.tensor_tensor(out=h_t[:, ti, :], in0=h_t[:, ti, :],
                                            in1=ari_t[:, ti, :], op=ALU.add)

            # ---------------- patch pooling: h = patchesT ----------------
            embS_v = embS_t
            cnt_v = cnt_t
            for ti in range(8):
                ps = pp.tile([128, 512], F32, tag="mm")
                for vc in range(2):
                    nc.tensor.matmul(ps[:], embS_v[:, vc, ti * 128:(ti + 1) * 128],
                                     cnt_v[:, vc, :],
                                     start=(vc == 0), stop=(vc == 1))
                nc.vector.tensor_copy(h_t[:, ti, :], ps[:])

            # ---------------- transformer layers ----------------
            for l in range(4):
                wqkv_t = wp.tile([128, 8, 384], F32R, tag="wqkv")
                for q in range(2):
                    nc.sync.dma_start(
                        wqkv_t[:, q * 4:(q + 1) * 4, :],
                        d["wqkv"][l].rearrange("p (kc x) -> p kc x", kc=8)
                        [:, q * 4:(q + 1) * 4, :])
                wo_t = wp.tile([128, 1024], F32R, tag="wo")
                nc.sync.dma_start(wo_t[:], d["wo"][l])
                wsq_t = wp.tile([128, 3], F32, tag="wsq")
                nc.sync.dma_start(wsq_t[:], d["wsq"][l])
                ngq_t = wp.tile([128, 3], F32, tag="ngq")
                nc.sync.dma_start(ngq_t[:], d["ngq"][l])
                bo8_t = wp.tile([128, 8], F32, tag="bo8")
                nc.sync.dma_start(bo8_t[:], d["bo8"][l])

                # ---- attention sublayer ----
                rsb, msb = stats(h_t, 8, 512)
                for m in range(3):
                    ps = pp.tile([128, 512], F32, tag="mm")
                    for kc in range(8):
                        nc.tensor.matmul(ps[:],
                                         wqkv_t[:, kc, m * 128:(m + 1) * 128],
                                         h_t[:, kc, :],
                                         start=(kc == 0), stop=(kc == 7))
                    fixup(ps, m, rsb, msb, wsq_t, ngq_t, qkv_t[:, m, :])
                # shift head-1 rows (partitions 64-127) down to base 0
                nc.sync.dma_start(qkvh2_t[:], qkv_t[64:128, :, :])

                for b in range(2):
                    for hh in range(2):
                        src = qkv_t if hh == 0 else qkvh2_t
                        qT = src[0:64, 0, b * 256:(b + 1) * 256]
                        kT = src[0:64, 1, b * 256:(b + 1) * 256]
                        vT = src[0:64, 2, b * 256:(b + 1) * 256]
                        em = tp.tile([128, 2, 256], F32R, tag="em")
                        for kt in range(2):
                            ps_s = pa.tile([128, 256], F32, tag="att")
                            nc.tensor.matmul(ps_s[:],
                                             kT[:, kt * 128:(kt + 1) * 128],
                                             qT[:], start=True, stop=True)
                            ex = tp.tile([128, 256], F32R, tag="ex")
                            nc.scalar.activation(ex[:], ps_s[:], AF.Exp,
                                                 scale=0.125)
                            nc.vector.tensor_tensor(
                                out=em[:, kt, :], in0=ex[:],
                                in1=masks_t[:, kt * 256:(kt + 1) * 256],
                                op=ALU.mult)
                        ps_d = pst.tile([1, 512], F32, tag="stat")
                        for kt in range(2):
                            nc.tensor.matmul(ps_d[:, :256], ones_t[:, 0:1],
                                             em[:, kt, :],
                                             start=(kt == 0), stop=(kt == 1))
                        rec = tps.tile([1, 256], F32R, tag="rec")
                        with nc.allow_low_precision(reason="f32r full width"):
                            nc.vector.reciprocal(rec[:], ps_d[:, :256])
                        ps_rb = pp.tile([128, 512], F32, tag="mm")
                        nc.tensor.matmul(ps_rb[:, :256], ones_t[0:1, :], rec[:],
                                         start=True, stop=True)
                        rec_b = tp.tile([128, 256], F32R, tag="recb")
                        nc.vector.tensor_copy(rec_b[:], ps_rb[:, :256])
                        vtok = tp.tile([128, 2, 64], F32R, tag="vtok")
                        for kt in range(2):
                            ps_t = pa.tile([128, 256], F32R, tag="att")
                            nc.tensor.transpose(ps_t[:, :64],
                                                vT[:, kt * 128:(kt + 1) * 128],
                                                ident_t[0:64, 0:64])
                            nc.vector.tensor_copy(vtok[:, kt, :], ps_t[:, :64])
                        ps_o = pa.tile([128, 256], F32, tag="att")
                        for kt in range(2):
                            nc.tensor.matmul(
                                ps_o[0:64, :], vtok[:, kt, :],
                                em[:, kt, :], start=(kt == 0), stop=(kt == 1))
                        if hh == 0:
                            nc.vector.tensor_tensor(
                                out=A_t[0:64, b * 256:(b + 1) * 256],
                                in0=ps_o[0:64, :],
                                in1=rec_b[0:64, :], op=ALU.mult)
                        else:
                            oh = tp.tile([64, 256], F32R, tag="oh")
                            nc.vector.tensor_tensor(
                                out=oh[:], in0=ps_o[0:64, :],
                                in1=rec_b[0:64, :], op=ALU.mult)
                            nc.sync.dma_start(
                                A_t[64:128, b * 256:(b + 1) * 256], oh[:])

                for ht in range(8):
                    ps = pp.tile([128, 512], F32, tag="mm")
                    nc.tensor.matmul(ps[:], wo_t[:, ht * 128:(ht + 1) * 128],
                                     A_t[:], start=True, stop=True)
                    nc.vector.tensor_scalar(out=aro_t[:, ht, :], in0=ps[:],
                                            scalar1=bo8_t[:, ht:ht + 1],
                                            scalar2=None, op0=ALU.add)
                allreduce_fp16(f"a{l}")
                resid_add()

                # ---- mlp sublayer ----
                w1_t = wp.tile([128, 8, 512], F32R, tag="w1")
                for q in range(2):
                    nc.sync.dma_start(
                        w1_t[:, q * 4:(q + 1) * 4, :],
                        d["w1"][l].rearrange("p (kc x) -> p kc x", kc=8)
                        [:, q * 4:(q + 1) * 4, :])
                w2_t = wp.tile([128, 4, 1024], F32R, tag="w2")
                for q in range(2):
                    nc.sync.dma_start(
                        w2_t[:, q * 2:(q + 1) * 2, :],
                        d["w2"][l].rearrange("p (kc x) -> p kc x", kc=4)
                        [:, q * 2:(q + 1) * 2, :])
                ws1_t = wp.tile([128, 4], F32, tag="ws1")
                nc.sync.dma_start(ws1_t[:], d["ws1"][l])
                ng1_t = wp.tile([128, 4], F32, tag="ng1")
                nc.sync.dma_start(ng1_t[:], d["ng1"][l])
                b28_t = wp.tile([128, 8], F32, tag="b28")
                nc.sync.dma_start(b28_t[:], d["b28"][l])

                rsb, msb = stats(h_t, 8, 512)
                for m in range(4):
                    ps = pp.tile([128, 512], F32, tag="mm")
                    for kc in range(8):
                        nc.tensor.matmul(ps[:],
                                         w1_t[:, kc, m * 128:(m + 1) * 128],
                                         h_t[:, kc, :],
                                         start=(kc == 0), stop=(kc == 7))
                    fixup(ps, m, rsb, msb, ws1_t, ng1_t, gu_t[:, m, :],
                          gelu=True)
                for ht in range(8):
                    ps = pp.tile([128, 512], F32, tag="mm")
                    for uc in range(4):
                        nc.tensor.matmul(ps[:],
                                         w2_t[:, uc, ht * 128:(ht + 1) * 128],
                                         gu_t[:, uc, :],
                                         start=(uc == 0), stop=(uc == 3))
                    nc.vector.tensor_scalar(out=aro_t[:, ht, :], in0=ps[:],
                                            scalar1=b28_t[:, ht:ht + 1],
                                            scalar2=None, op0=ALU.add)
                allreduce_fp16(f"m{l}")
                resid_add()

            # ---------------- final norm -> pf (in place into h) ----------
            rsb, msb = stats(h_t, 8, 512)
            for ti in range(8):
                t1 = tp.tile([128, 512], F32R, tag="fx1")
                nc.vector.tensor_tensor(out=t1[:], in0=h_t[:, ti, :],
                                        in1=rsb[:], op=ALU.mult)
                t2 = tp.tile([128, 512], F32R, tag="fx2")
                nc.vector.tensor_tensor(out=t2[:], in0=t1[:], in1=msb[:],
                                        op=ALU.subtract)
                nc.vector.tensor_scalar(out=h_t[:, ti, :], in0=t2[:],
                                        scalar1=fng_t[:, ti:ti + 1],
                                        scalar2=fnb_t[:, ti:ti + 1],
                                        op0=ALU.mult, op1=ALU.add)
            # kvn = ln(pf)*cag + cab   (into the w1 weight slot)
            kvn_t = wp.tile([128, 8, 512], F32R, tag="w1")
            rsb, msb = stats(h_t, 8, 512)
            for ti in range(8):
                t1 = tp.tile([128, 512], F32R, tag="fx1")
                nc.vector.tensor_tensor(out=t1[:], in0=h_t[:, ti, :],
                                        in1=rsb[:], op=ALU.mult)
                t2 = tp.tile([128, 512], F32R, tag="fx2")
                nc.vector.tensor_tensor(out=t2[:], in0=t1[:], in1=msb[:],
                                        op=ALU.subtract)
                nc.vector.tensor_scalar(out=kvn_t[:, ti, :], in0=t2[:],
                                        scalar1=cag_t[:, ti:ti + 1],
                                        scalar2=cab_t[:, ti:ti + 1],
                                        op0=ALU.mult, op1=ALU.add)

            # ---------------- qn = ln(embT)*cag + cab ----------------
            embT_t = wp.tile([128, 8, 256], F32R, tag="w2")
            nc.sync.dma_start(embT_t[:], d["embT"][:].rearrange(
                "p (kc x) -> p kc x", kc=8))
            qn_t = sbp.tile([128, 8, 256], F32R, tag="gu")
            rsb, msb = stats(embT_t, 8, 256)
            for ti in range(8):
                t1 = tp.tile([128, 512], F32R, tag="fx1")
                nc.vector.tensor_tensor(out=t1[:, :256], in0=embT_t[:, ti, :],
                                        in1=rsb[:, :256], op=ALU.mult)
                t2 = tp.tile([128, 512], F32R, tag="fx2")
                nc.vector.tensor_tensor(out=t2[:, :256], in0=t1[:, :256],
                                        in1=msb[:, :256], op=ALU.subtract)
                nc.vector.tensor_scalar(out=qn_t[:, ti, :], in0=t2[:, :256],
                                        scalar1=cag_t[:, ti:ti + 1],
                                        scalar2=cab_t[:, ti:ti + 1],
                                        op0=ALU.mult, op1=ALU.add)

            # ---------------- CA projections ----------------
            wcat_t = wp.tile([128, 3, 8, 128], F32R, tag="wqkv")
            for i, nm in enumerate(("wq", "wk", "wv")):
                nc.sync.dma_start(wcat_t[:, i], d[nm][:].rearrange(
                    "p (kc x) -> p kc x", kc=8))
            wq_v = wcat_t[:, 0]
            wk_v = wcat_t[:, 1]
            wv_v = wcat_t[:, 2]

            kT_t = sbp.tile([128, 512], F32R, tag="kT")
            vT_t = sbp.tile([128, 512], F32R, tag="vT")
            qT_t = sbp.tile([128, 256], F32R, tag="qT")
            for (w_v, bias_t, out_t, src, width) in (
                (wk_v, bk_t, kT_t, kvn_t, 512),
                (wv_v, bv_t, vT_t, kvn_t, 512),
                (wq_v, bq_t, qT_t, qn_t, 256),
            ):
                ps = pp.tile([128, 512], F32, tag="mm")
                for kc in range(8):
                    nc.tensor.matmul(ps[:, :width], w_v[:, kc, :],
                                     src[:, kc, :],
                                     start=(kc == 0), stop=(kc == 7))
                nc.vector.tensor_scalar(out=out_t[:, :width], in0=ps[:, :width],
                                        scalar1=bias_t[:], scalar2=None,
                                        op0=ALU.add)

            # ---------------- CA attention (1 head, dh=128, both batches) ----
            O_t = sbp.tile([128, 512], F32R, tag="O")
            for b in range(2):
                em = tp.tile([128, 2, 256], F32R, tag="em")
                for kt in range(2):
                    ps_s = pa.tile([128, 256], F32, tag="att")
                    nc.tensor.matmul(
                        ps_s[:], kT_t[:, b * 256 + kt * 128: b * 256 + (kt + 1) * 128],
                        qT_t[:], start=True, stop=True)
                    nc.scalar.activation(em[:, kt, :], ps_s[:], AF.Exp,
                                         scale=float(1.0 / np.sqrt(128.0)))
                ps_d = pst.tile([1, 512], F32, tag="stat")
                for kt in range(2):
                    nc.tensor.matmul(ps_d[:, :256], ones_t[:, 0:1], em[:, kt, :],
                                     start=(kt == 0), stop=(kt == 1))
                rec = tps.tile([1, 256], F32R, tag="rec")
                with nc.allow_low_precision(reason="f32r full width"):
                    nc.vector.reciprocal(rec[:], ps_d[:, :256])
                ps_rb = pp.tile([128, 512], F32, tag="mm")
                nc.tensor.matmul(ps_rb[:, :256], ones_t[0:1, :], rec[:],
                                 start=True, stop=True)
                rec_b = tp.tile([128, 256], F32R, tag="recb")
                nc.vector.tensor_copy(rec_b[:], ps_rb[:, :256])
                vtok = tp.tile([128, 2, 128], F32R, tag="vtokca")
                for kt in range(2):
                    ps_t = pa.tile([128, 256], F32R, tag="att")
                    nc.tensor.transpose(
                        ps_t[:, :128],
                        vT_t[:, b * 256 + kt * 128: b * 256 + (kt + 1) * 128],
                        ident_t[:])
                    nc.vector.tensor_copy(vtok[:, kt, :], ps_t[:, :128])
                ps_o = pa.tile([128, 256], F32, tag="att")
                for kt in range(2):
                    nc.tensor.matmul(ps_o[:], vtok[:, kt, :], em[:, kt, :],
                                     start=(kt == 0), stop=(kt == 1))
                nc.vector.tensor_tensor(out=O_t[:, b * 256:(b + 1) * 256],
                                        in0=ps_o[:], in1=rec_b[:], op=ALU.mult)

            # ---------------- logits partials + AR ----------------
            cawoT_t = wp.tile([128, 8, 128], F32R, tag="wo")
            nc.sync.dma_start(cawoT_t[:], d["cawoT"][:].rearrange(
                "p (kc x) -> p kc x", kc=8))
            cawoT_v = cawoT_t
            headw_t = sbp.tile([128, 8, 256], F32R, tag="sq")
            nc.sync.dma_start(headw_t[:], d["headw"][:].rearrange(
                "p (kc x) -> p kc x", kc=8))
            headw_v = headw_t
            w2c_t = sbp.tile([128, 256], F32R, tag="w2c")
            ps = pp.tile([128, 512], F32, tag="mm")
            for kc in range(8):
                nc.tensor.matmul(ps[:, :256], cawoT_v[:, kc, :],
                                 headw_v[:, kc, :],
                                 start=(kc == 0), stop=(kc == 7))
            nc.vector.tensor_copy(w2c_t[:], ps[:, :256])

            lp_t = sbp.tile([128, 2, 512], F32, tag="qkv")
            for lt in range(2):
                ps = pp.tile([128, 512], F32, tag="mm")
                nc.tensor.matmul(ps[:], w2c_t[:, lt * 128:(lt + 1) * 128],
                                 O_t[:], start=True, stop=True)
                nc.vector.tensor_copy(lp_t[:, lt, :], ps[:])
            lbin = dp.tile([128, 1024], F32, tag="lci")
            lbout = dp.tile([128, 1024], F32, addr_space="Shared", tag="lco")
            nc.sync.dma_start(lbin[:], lp_t[:])
            nc.gpsimd.collective_compute(
                "AllReduce", ALU.add, replica_groups=RG8,
                ins=[lbin[:].opt()], outs=[lbout[:].opt()])
            lar_t = sbp.tile([128, 2, 512], F32, tag="aro")
            nc.sync.dma_start(lar_t[:], lbout[:])

            # emb @ head_w term + head bias
            emT_v = embT_t  # [128, 8, 256] f32r view (still loaded)
            out_t = sbp.tile([128, 2, 512], F32, tag="ari")
            for lt in range(2):
                ps_e = pp.tile([128, 512], F32, tag="mm")
                for kc in range(8):
                    nc.tensor.matmul(ps_e[:, :256],
                                     headw_v[:, kc, lt * 128:(lt + 1) * 128],
                                     emT_v[:, kc, :],
                                     start=(kc == 0), stop=(kc == 7))
                et = tp.tile([128, 256], F32, tag="et")
                nc.vector.tensor_copy(et[:], ps_e[:, :256])
                tb = tp.tile([128, 512], F32, tag="tb")
                nc.vector.tensor_scalar(out=tb[:], in0=lar_t[:, lt, :],
                                        scalar1=headb_t[:, lt:lt + 1],
                                        scalar2=None, op0=ALU.add)
                for b in range(2):
                    nc.vector.tensor_tensor(
                        out=out_t[:, lt, b * 256:(b + 1) * 256],
                        in0=tb[:, b * 256:(b + 1) * 256], in1=et[:],
                        op=ALU.add)
            nc.sync.dma_start(out_d[:], out_t[:])

    nc.compile()
    nc.m = get_hw_module(nc.m)
    return nc


# --------------------------------------------------------------------------
# host side
# --------------------------------------------------------------------------
def _shuf(M):
    """[K, X] -> [128, (K//128)*X] laid out as [p, kc, x]."""
    K, X = M.shape
    return np.ascontiguousarray(
        M.reshape(K // 128, 128, X).transpose(1, 0, 2).reshape(128, -1))


def _prep(inputs):
    f = lambda k: np.asarray(inputs[k], np.float32)
    byte_seq = np.asarray(inputs["byte_seq"])
    bd = np.asarray(inputs["patch_boundaries"])
    emb = f("emb")

    # patch histogram matrix
    pos = np.arange(S)
    pid = np.stack([np.searchsorted(bd[b], pos, side="right") for b in range(B)])
    pid = np.clip(pid, 0, P - 1)
    Cn = np.zeros((B, P, V), np.float32)
    for b in range(B):
        np.add.at(Cn[b], (pid[b], byte_seq[b]), 1.0)
    cnts = Cn.sum(-1)
    Cn /= np.maximum(cnts, 1.0)[..., None]
    cnt_all = np.concatenate([Cn[0].T, Cn[1].T], axis=1)  # [V, 512]

    g1, b1a = f("g_ln1_g"), f("g_ln1_b")
    g2, b2a = f("g_ln2_g"), f("g_ln2_b")
    Wqkv, bqkv = f("g_wqkv"), f("g_bqkv")
    Wo, bo = f("g_wo"), f("g_bo")
    W1, b1 = f("g_w1"), f("g_b1")
    W2, b2 = f("g_w2"), f("g_b2")

    Wq_f = g1[:, :, None] * Wqkv                       # [L, H, 3H]
    biasq = np.einsum("lh,lho->lo", b1a, Wqkv) + bqkv  # [L, 3H]
    wsumq = Wq_f.sum(1)                                # [L, 3H]
    W1_f = g2[:, :, None] * W1
    bias1 = np.einsum("lh,lho->lo", b2a, W1) + b1
    wsum1 = W1_f.sum(1)

    ca_wqkv, ca_bqkv = f("ca_wqkv"), f("ca_bqkv")
    ca_wo, ca_bo = f("ca_wo"), f("ca_bo")
    head_w, head_b = f("head_w"), f("head_b")
    headb_full = head_b + ca_bo @ head_w               # [256]

    masks = np.zeros((128, 2, 256), np.float32)
    for kt in range(2):
        ktg = kt * 128 + np.arange(128)
        masks[:, kt, :] = (ktg[:, None] <= np.arange(256)[None, :])

    shared = {
        "headw": _shuf(head_w),
        "headb": np.ascontiguousarray(headb_full.reshape(2, 128).T),
        "embT": _shuf(np.ascontiguousarray(emb.T)),
        "embS": _shuf(emb),
        "cnt": _shuf(cnt_all),
        "masks": np.ascontiguousarray(masks.reshape(128, 512)),
        "ones": np.ones((128, 128), np.float32),
        "ident": np.eye(128, dtype=np.float32),
        "fng": np.ascontiguousarray(f("fn_g").reshape(8, 128).T),
        "fnb": np.ascontiguousarray(f("fn_b").reshape(8, 128).T),
        "cag": np.ascontiguousarray(f("ca_ln_g").reshape(8, 128).T),
        "cab": np.ascontiguousarray(f("ca_ln_b").reshape(8, 128).T),
        "bo8": np.ascontiguousarray(
            bo.reshape(L, 8, 128).transpose(0, 2, 1) / NC),
        "b28": np.ascontiguousarray(
            b2.reshape(L, 8, 128).transpose(0, 2, 1) / NC),
    }

    in_maps = []
    for c in range(NC):
        cols = np.concatenate([np.arange(c * 128, (c + 1) * 128) + k * H
                               for k in range(3)])
        m = dict(shared)
        m["wqkv"] = np.stack([_shuf(Wq_f[l][:, cols]) for l in range(L)])
        m["wsq"] = np.ascontiguousarray(
            wsumq[:, cols].reshape(L, 3, 128).transpose(0, 2, 1))
        m["ngq"] = np.ascontiguousarray(
            (-biasq[:, cols]).reshape(L, 3, 128).transpose(0, 2, 1))
        m["wo"] = np.ascontiguousarray(Wo[:, c * 128:(c + 1) * 128, :])
        m["w1"] = np.stack([_shuf(W1_f[l][:, c * 512:(c + 1) * 512])
                            for l in range(L)])
        m["ws1"] = np.ascontiguousarray(
            wsum1[:, c * 512:(c + 1) * 512].reshape(L, 4, 128)
            .transpose(0, 2, 1))
        m["ng1"] = np.ascontiguousarray(
            (-bias1[:, c * 512:(c + 1) * 512]).reshape(L, 4, 128)
            .transpose(0, 2, 1))
        m["w2"] = np.stack([_shuf(W2[l][c * 512:(c + 1) * 512, :])
                            for l in range(L)])
        m["wq"] = _shuf(ca_wqkv[:, c * 128:(c + 1) * 128])
        m["wk"] = _shuf(ca_wqkv[:, H + c * 128: H + (c + 1) * 128])
        m["wv"] = _shuf(ca_wqkv[:, 2 * H + c * 128: 2 * H + (c + 1) * 128])
        m["bq"] = np.ascontiguousarray(
            ca_bqkv[c * 128:(c + 1) * 128, None])
        m["bk"] = np.ascontiguousarray(
            ca_bqkv[H + c * 128: H + (c + 1) * 128, None])
        m["bv"] = np.ascontiguousarray(
            ca_bqkv[2 * H + c * 128: 2 * H + (c + 1) * 128, None])
        m["cawoT"] = _shuf(np.ascontiguousarray(
            ca_wo[c * 128:(c + 1) * 128, :].T))
        in_maps.append(m)
    return in_maps, byte_seq


def run_device(inputs, trace=False):
    if "nc" not in _CACHE:
        _CACHE["nc"] = _trace()
    nc = _CACHE["nc"]
    in_maps, byte_seq = _prep(inputs)
    res = run_bass_kernel_spmd(nc, in_maps, core_ids=list(range(NC)),
                               trace=trace)
    ltab = res.results[0]["ltab"]                     # [128, 1024]
    ltab = ltab.reshape(128, 2, 512).transpose(1, 0, 2).reshape(256, 512)
    out = np.empty((B, S, V), np.float32)
    for b in range(B):
        tab_b = ltab[:, b * 256:(b + 1) * 256]        # [lc, v]
        out[b] = tab_b.T[byte_seq[b]]                 # [S, 256]
    return out, res


def kernel(**inputs) -> np.ndarray:
    out, _ = run_device(inputs, trace=False)
    return out
